# revision 3
# baseline (speedup 1.0000x reference)
"""Trainium2 Bass kernel for nn_DeepSupervisionBoundaryDoULoss.

kernel(**inputs) takes the FULL unsharded inputs (logits0/1/2, targets,
valid_mask) and returns the full scalar loss (float32).

Strategy: data-parallel over the 32 (b,n) pairs -> 4 pairs per core x 8 cores.
Each core streams its slice (~9.25 MiB) once; DMA is the roofline (~29us at
~340 GB/s/core). Engine split keeps every engine under the DMA shadow:

  - Pool(gpsimd): int32->fp8e4 casting DMAs for targets (deinterleaved rows
    A/B/C/D as [128, half, parity, 516] w/ 2-col zero pads), pad memsets.
  - Sync(SP):     consts + all logits DMAs (big first, scale-2 last).
  - ACT(scalar):  sigmoid (f32->bf16) per scale; interior-count relu
    (bias=-4, accum_out) read straight from multi-bank PSUM; out-DMA issue.
  - DVE(vector):  inter = sum(p*t) and z = sum(p^2) via scalar_tensor_tensor
    with accum_out (bf16 x fp8 / bf16 x bf16); t2 extraction copy.
  - PE(tensor):   3x3-cross conv as fp8 DoubleRow matmuls: per tile ONE DR
    fuses [I|band]@[center|other-parity] and ONE DR fuses [I|I]@[left|right]
    (overlapping strided ifmap APs), accumulated in PSUM f32.

Per-pair stats (4 counts + 3 inter + 3 z) land in one [128,10] f32 block,
DMA'd out per pair; the host reduces partitions and finishes alpha/dou/mean
plus a seam correction for the 4 rows/pair the on-chip conv cannot see
(identical structure to the row-deinterleave: rows 255/256 at s0, 127/128 at
s1).
"""

from contextlib import ExitStack

import numpy as np

N_PAIRS = 4
N_CORES = 8
H0, H1, H2 = 512, 256, 128
N_SCALES = 3
SMOOTH = 1e-5

# per-pair stats columns
C_CNT0A, C_CNT0B, C_CNT1, C_CNT2 = 0, 1, 2, 3
C_INT0, C_INT1, C_INT2 = 4, 5, 6
C_Z0, C_Z1, C_Z2 = 7, 8, 9
COLS_PER_PAIR = 10

# consts layout (fp8), free-dim offsets
W_IB2M = 0      # [2,128]  DR weights [I | B2M]
W_B2PI = 256    # [2,128]  [B2P | I]
W_II = 512      # [2,128]  [I | I]
W_ITRI = 768    # [2,128]  [I | TRI]
W_I = 1024      # [128]    plain identity
W_EVEN = 1152   # [64]     even-partition selector
N_CONST = 1216

_NC_CACHE = {}


def make_consts():
    import ml_dtypes

    ident = np.eye(128, dtype=np.float32)
    b2m = np.zeros((128, 128), np.float32)  # q in {i-1, i}
    b2p = np.zeros((128, 128), np.float32)  # q in {i, i+1}
    tri = np.zeros((128, 128), np.float32)  # q in {i-1, i, i+1}
    for i in range(128):
        for dq in (-1, 0):
            if 0 <= i + dq < 128:
                b2m[i + dq, i] = 1.0
        for dq in (0, 1):
            if 0 <= i + dq < 128:
                b2p[i + dq, i] = 1.0
        for dq in (-1, 0, 1):
            if 0 <= i + dq < 128:
                tri[i + dq, i] = 1.0
    even = np.zeros((128, 64), np.float32)
    for i in range(64):
        even[2 * i, i] = 1.0
    c = np.zeros((128, N_CONST), np.float32)
    c[:, 0:128], c[:, 128:256] = ident, b2m
    c[:, 256:384], c[:, 384:512] = b2p, ident
    c[:, 512:640], c[:, 640:768] = ident, ident
    c[:, 768:896], c[:, 896:1024] = ident, tri
    c[:, 1024:1152] = ident
    c[:, 1152:1216] = even
    return c.astype(ml_dtypes.float8_e4m3fn)


def build_kernel(n_pairs=N_PAIRS):
    import concourse.tile as tile
    from concourse import bacc, mybir
    from bass_rust import AP

    F32 = mybir.dt.float32
    F8 = mybir.dt.float8e4
    BF16 = mybir.dt.bfloat16
    I32 = mybir.dt.int32
    ALU = mybir.AluOpType
    ACTF = mybir.ActivationFunctionType
    DRM = mybir.MatmulPerfMode.DoubleRow

    ncols = n_pairs * COLS_PER_PAIR
    nc = bacc.Bacc("TRN2", target_bir_lowering=False, debug=False)

    logits0 = nc.dram_tensor("logits0", [n_pairs, H0, H0], F32, kind="ExternalInput").ap()
    logits1 = nc.dram_tensor("logits1", [n_pairs, H1, H1], F32, kind="ExternalInput").ap()
    logits2 = nc.dram_tensor("logits2", [n_pairs, H2, H2], F32, kind="ExternalInput").ap()
    targets = nc.dram_tensor("targets", [n_pairs, H0, H0], I32, kind="ExternalInput").ap()
    consts8 = nc.dram_tensor("consts_f8", [128, N_CONST], F8, kind="ExternalInput").ap()
    out = nc.dram_tensor("out", [128, ncols], F32, kind="ExternalOutput").ap()

    with tile.TileContext(nc) as tc, ExitStack() as ctx:
        singles = ctx.enter_context(tc.tile_pool(name="singles", bufs=1))
        tpool = ctx.enter_context(tc.tile_pool(name="tpool", bufs=4))
        lpool = ctx.enter_context(tc.tile_pool(name="lpool", bufs=4))
        ppool = ctx.enter_context(tc.tile_pool(name="ppool", bufs=4))
        spool = ctx.enter_context(tc.tile_pool(name="spool", bufs=2))
        ps0p = ctx.enter_context(tc.tile_pool(name="ps0p", bufs=2, space="PSUM"))
        ps1p = ctx.enter_context(tc.tile_pool(name="ps1p", bufs=2, space="PSUM"))
        ps2p = ctx.enter_context(tc.tile_pool(name="ps2p", bufs=2, space="PSUM"))

        cb = singles.tile([128, N_CONST], F8)
        nc.sync.dma_start(out=cb, in_=consts8)
        stats = singles.tile([128, ncols], F32)
        neg4 = singles.tile([128, 1], F32)
        nc.vector.memset(neg4, -4.0)

        def wdr(off):
            return cb[:, off : off + 256].rearrange("p (two m) -> p two m", two=2)

        # ---- phase A: all input DMAs queued up front ----
        t0s, l0s, l1s, l2s = [], [], [], []
        for p in range(n_pairs):
            t0 = tpool.tile([128, 2, 2, 516], F8, tag="t0", name=f"t0_{p}")
            for half in range(2):
                nc.gpsimd.dma_start(
                    out=t0[:, half, :, 2:514],
                    in_=targets[p, half * 256 : (half + 1) * 256].rearrange(
                        "(r parity) c -> r parity c", parity=2
                    ),
                )
            nc.gpsimd.memset(t0[:, :, :, 0:2], 0.0)
            nc.gpsimd.memset(t0[:, :, :, 514:516], 0.0)
            t0s.append(t0)
        for p in range(n_pairs):
            l0 = lpool.tile([128, 2, 2, 512], F32, tag="l0", name=f"l0_{p}")
            for half in range(2):
                nc.sync.dma_start(
                    out=l0[:, half],
                    in_=logits0[p, half * 256 : (half + 1) * 256].rearrange(
                        "(r parity) c -> r parity c", parity=2
                    ),
                )
            l0s.append(l0)
        for p in range(n_pairs):
            l1 = lpool.tile([128, 2, 256], F32, tag="l1", name=f"l1_{p}")
            nc.sync.dma_start(
                out=l1, in_=logits1[p].rearrange("(g r) c -> r g c", g=2)
            )
            l1s.append(l1)
        for p in range(n_pairs):
            l2 = lpool.tile([128, 128], F32, tag="l2", name=f"l2_{p}")
            nc.sync.dma_start(out=l2, in_=logits2[p])
            l2s.append(l2)

        def windows(t, off, bstride, nb, istride, n):
            pstride = 1
            for s in t.tensor.shape[1:]:
                pstride *= s
            return AP(tensor=t.tensor, offset=off,
                      ap=[[pstride, 128], [bstride, nb], [istride, n]])

        # ---- per-pair compute, software-pipelined: sigmoids+PE+DVE for pair
        # p are emitted before ACT counts of pair p-1 so ACT never head-of-
        # line blocks on the tensor engine.
        def emit_front(p):
            t0, l0, l1, l2 = t0s[p], l0s[p], l1s[p], l2s[p]
            st = stats[:, p * COLS_PER_PAIR : (p + 1) * COLS_PER_PAIR]

            # scale 0
            p0 = ppool.tile([128, 2, 2, 512], BF16, tag="p0", name=f"p0_{p}")
            nc.scalar.activation(out=p0, in_=l0, func=ACTF.Sigmoid)
            scr0 = spool.tile([128, 2, 2, 512], BF16, tag="scr0", name=f"scr0_{p}")
            nc.vector.scalar_tensor_tensor(
                out=scr0, in0=p0, scalar=1.0, in1=t0[:, :, :, 2:514],
                op0=ALU.mult, op1=ALU.mult,
                accum_out=st[:, C_INT0 : C_INT0 + 1],
            )
            nc.vector.scalar_tensor_tensor(
                out=scr0, in0=p0, scalar=1.0, in1=p0,
                op0=ALU.mult, op1=ALU.mult,
                accum_out=st[:, C_Z0 : C_Z0 + 1],
            )
            pss = []
            for half in range(2):
                ps = ps0p.tile([128, 2, 512], F32, tag="ps0", name=f"ps0_{p}_{half}")
                for parity in range(2):
                    dst = ps[:, parity, :]
                    w1 = wdr(W_IB2M if parity == 0 else W_B2PI)
                    nc.tensor.matmul(dst, w1, t0[:, half, :, 2:514],
                                     start=True, stop=False, perf_mode=DRM)
                    off = (half * 2 + parity) * 516 + 1
                    nc.tensor.matmul(dst, wdr(W_II), windows(t0, off, 2, 2, 1, 512),
                                     start=False, stop=True, perf_mode=DRM)
                pss.append(ps)

            # scale 1: rows = A/C planes, cols strided 2 (pads included)
            p1 = ppool.tile([128, 2, 256], BF16, tag="p1", name=f"p1_{p}")
            nc.scalar.activation(out=p1, in_=l1, func=ACTF.Sigmoid)
            scr1 = spool.tile([128, 2, 256], BF16, tag="scr1", name=f"scr1_{p}")
            t1c = t0[:, :, 0, 2:514:2]
            nc.vector.scalar_tensor_tensor(
                out=scr1, in0=p1, scalar=1.0, in1=t1c,
                op0=ALU.mult, op1=ALU.mult,
                accum_out=st[:, C_INT1 : C_INT1 + 1],
            )
            nc.vector.scalar_tensor_tensor(
                out=scr1, in0=p1, scalar=1.0, in1=p1,
                op0=ALU.mult, op1=ALU.mult,
                accum_out=st[:, C_Z1 : C_Z1 + 1],
            )
            ps1 = ps1p.tile([128, 2, 256], F32, tag="ps1", name=f"ps1_{p}")
            for g in range(2):
                dst = ps1[:, g, :]
                off = (g * 2) * 516
                nc.tensor.matmul(dst, wdr(W_ITRI), windows(t0, off, 2, 2, 2, 256),
                                 start=True, stop=False, perf_mode=DRM)
                nc.tensor.matmul(dst, cb[:, W_I : W_I + 128],
                                 t0[:, g, 0, 4:516:2], start=False, stop=True)

            # scale 2: t2 = t0[::4, ::4] via even-partition selection
            ps2 = ps2p.tile([128, 2, 128], F32, tag="ps2", name=f"ps2_{p}")
            for g in range(2):
                nc.tensor.matmul(ps2[g * 64 : (g + 1) * 64, 0, :],
                                 cb[:, W_EVEN : W_EVEN + 64],
                                 t0[:, g, 0, 2:514:4], start=True, stop=True)
            t2 = tpool.tile([128, 130], F8, tag="t2", name=f"t2_{p}")
            nc.gpsimd.memset(t2[:, 0:1], 0.0)
            nc.gpsimd.memset(t2[:, 129:130], 0.0)
            nc.vector.tensor_copy(t2[:, 1:129], ps2[:, 0, :])

            p2 = ppool.tile([128, 128], BF16, tag="p2", name=f"p2_{p}")
            nc.scalar.activation(out=p2, in_=l2, func=ACTF.Sigmoid)
            scr2 = spool.tile([128, 128], BF16, tag="scr2", name=f"scr2_{p}")
            nc.vector.scalar_tensor_tensor(
                out=scr2, in0=p2, scalar=1.0, in1=t2[:, 1:129],
                op0=ALU.mult, op1=ALU.mult,
                accum_out=st[:, C_INT2 : C_INT2 + 1],
            )
            nc.vector.scalar_tensor_tensor(
                out=scr2, in0=p2, scalar=1.0, in1=p2,
                op0=ALU.mult, op1=ALU.mult,
                accum_out=st[:, C_Z2 : C_Z2 + 1],
            )
            dst = ps2[:, 1, :]
            nc.tensor.matmul(dst, wdr(W_ITRI), windows(t2, 0, 1, 2, 1, 128),
                             start=True, stop=False, perf_mode=DRM)
            nc.tensor.matmul(dst, cb[:, W_I : W_I + 128], t2[:, 2:130],
                             start=False, stop=True)
            return pss, ps1, ps2

        def emit_back(p, pss, ps1, ps2):
            st = stats[:, p * COLS_PER_PAIR : (p + 1) * COLS_PER_PAIR]
            cnt0 = spool.tile([128, 2, 512], F8, tag="cnt0", name=f"cnt0_{p}")
            for half in range(2):
                nc.scalar.activation(
                    out=cnt0, in_=pss[half], func=ACTF.Relu,
                    bias=neg4[:, 0:1], scale=1.0,
                    accum_out=st[:, C_CNT0A + half : C_CNT0A + half + 1],
                )
            cnt1 = spool.tile([128, 2, 256], F8, tag="cnt1", name=f"cnt1_{p}")
            nc.scalar.activation(
                out=cnt1, in_=ps1, func=ACTF.Relu, bias=neg4[:, 0:1], scale=1.0,
                accum_out=st[:, C_CNT1 : C_CNT1 + 1],
            )
            cnt2 = spool.tile([128, 128], F8, tag="cnt2", name=f"cnt2_{p}")
            nc.scalar.activation(
                out=cnt2, in_=ps2[:, 1, :], func=ACTF.Relu,
                bias=neg4[:, 0:1], scale=1.0,
                accum_out=st[:, C_CNT2 : C_CNT2 + 1],
            )
            nc.scalar.dma_start(
                out=out[:, p * COLS_PER_PAIR : (p + 1) * COLS_PER_PAIR], in_=st
            )

        pending = None
        for p in range(n_pairs):
            front = emit_front(p)
            if pending is not None:
                emit_back(pending[0], *pending[1])
            pending = (p, front)
        emit_back(pending[0], *pending[1])

    nc.compile()
    return nc


def get_kernel():
    if "nc" not in _NC_CACHE:
        _NC_CACHE["nc"] = build_kernel(N_PAIRS)
    return _NC_CACHE["nc"]


def seam_interior_counts(tg_pair):
    """Interior pixels in the seam rows the kernel cannot see (per scale)."""
    tg_pair = np.asarray(tg_pair)
    out = []
    for h in (H0, H1):
        step = H0 // h
        t = tg_pair[::step, ::step].astype(np.float64)
        pad = np.pad(t, 1)
        cnt = 0
        for r in (h // 2 - 1, h // 2):
            pr = r + 1
            nsum = (
                pad[pr, 1:-1] + pad[pr - 1, 1:-1] + pad[pr + 1, 1:-1]
                + pad[pr, 0:-2] + pad[pr, 2:]
            )
            cnt += int((nsum == 5.0).sum())
        out.append(float(cnt))
    out.append(0.0)
    return out


def combine_stats(all_core_outs, valid_mask, targets, n_pairs=N_PAIRS):
    vm = (np.asarray(valid_mask, np.float32).reshape(-1) >= 0.5).astype(np.float64)
    tg = np.asarray(targets).reshape(-1, H0, H0)
    n_total = vm.shape[0]
    per = np.zeros((N_SCALES, n_total), np.float64)
    for core, st in enumerate(all_core_outs):
        st = np.asarray(st, np.float64).sum(axis=0)  # reduce 128 partitions
        for j in range(n_pairs):
            g = core * n_pairs + j
            c = st[j * COLS_PER_PAIR : (j + 1) * COLS_PER_PAIR]
            seam = seam_interior_counts(tg[g])
            tgg = tg[g].astype(np.float64)
            host_S = [tgg.sum(), tgg[::2, ::2].sum(), tgg[::4, ::4].sum()]
            interior = [c[C_CNT0A] + c[C_CNT0B], c[C_CNT1], c[C_CNT2]]
            inter_v = [c[C_INT0], c[C_INT1], c[C_INT2]]
            z_v = [c[C_Z0], c[C_Z1], c[C_Z2]]
            for s in range(N_SCALES):
                S = host_S[s]
                C = S - (interior[s] + seam[s])
                alpha = min(2.0 * (1.0 - (C + SMOOTH) / (S + SMOOTH)) - 1.0, 0.8)
                dou = (z_v[s] + S - 2.0 * inter_v[s] + SMOOTH) / (
                    z_v[s] + S - (1.0 + alpha) * inter_v[s] + SMOOTH
                )
                per[s, g] = dou if S > 0 else 0.0
    cnt = vm.sum()
    ws = np.array([1.0, 0.5, 0.25])
    ws = ws / ws.sum()
    loss = 0.0
    for s in range(N_SCALES):
        ls = (per[s] * vm).sum() / cnt if cnt > 0 else 0.0
        loss += ws[s] * ls
    return np.float32(loss)


def make_in_maps(inputs):
    l0 = np.ascontiguousarray(np.asarray(inputs["logits0"], np.float32).reshape(-1, H0, H0))
    l1 = np.ascontiguousarray(np.asarray(inputs["logits1"], np.float32).reshape(-1, H1, H1))
    l2 = np.ascontiguousarray(np.asarray(inputs["logits2"], np.float32).reshape(-1, H2, H2))
    tg = np.ascontiguousarray(np.asarray(inputs["targets"], np.int32).reshape(-1, H0, H0))
    consts = np.asarray(make_consts())
    in_maps = []
    for core in range(N_CORES):
        lo, hi = core * N_PAIRS, (core + 1) * N_PAIRS
        in_maps.append({
            "logits0": np.ascontiguousarray(l0[lo:hi]),
            "logits1": np.ascontiguousarray(l1[lo:hi]),
            "logits2": np.ascontiguousarray(l2[lo:hi]),
            "targets": np.ascontiguousarray(tg[lo:hi]),
            "consts_f8": consts,
        })
    return in_maps


def run_cores(inputs, **spmd_kwargs):
    from concourse.bass_utils import run_bass_kernel_spmd

    nc = get_kernel()
    in_maps = make_in_maps(inputs)
    return run_bass_kernel_spmd(nc, in_maps, core_ids=list(range(N_CORES)), **spmd_kwargs)


def kernel(**inputs) -> np.ndarray:
    res = run_cores(inputs)
    outs = [res.results[c]["out"] for c in range(N_CORES)]
    return combine_stats(outs, inputs["valid_mask"], inputs["targets"])


# revision 7
# speedup vs baseline: 1.0319x; 1.0319x over previous
"""Trainium2 Bass kernel for nn_DeepSupervisionBoundaryDoULoss.

kernel(**inputs) takes the FULL unsharded inputs (logits0/1/2, targets,
valid_mask) and returns the full scalar loss (float32).

Strategy: data-parallel over the 32 (b,n) pairs -> 4 pairs per core x 8 cores.
Each core streams its slice (~9.25 MiB) once; DMA is the roofline (~29us at
~340 GB/s/core). Compute is emitted SCALE-MAJOR (all pairs s0, then s1, then
s2) so engine streams never head-of-line block on late DMA data, and the tail
ops are the tiny scale-2 ones.

  - Pool(gpsimd): int32->fp8e4 casting DMAs for targets (deinterleaved rows
    A/B/C/D as [128, half, parity, 516] w/ 2-col zero pads), pad memsets.
  - Sync(SP):     consts + logits0 DMAs + per-pair stats out-DMAs.
  - ACT(scalar):  logits1/2 DMA issue; sigmoid (f32->bf16); z = sum(p^2) via
    Square activation with accum_out.
  - DVE(vector):  inter = sum(p*t) via scalar_tensor_tensor (bf16 x fp8);
    interior counts via stt-relu ((nsum-4) max 0) read from PSUM with
    accum_out; t2 extraction copy.
  - PE(tensor):   3x3-cross conv as fp8 DoubleRow matmuls: per tile ONE DR
    fuses [I|band]@[center|other-parity] and ONE DR fuses [I|I]@[left|right]
    (overlapping strided ifmap APs), accumulated in PSUM f32.

Stats layout [128, 10] f32 per pair (partition-reduced on host): the host
finishes alpha/dou/weighted mean plus a seam correction for the 4 rows/pair
the on-chip conv cannot see (rows 255/256 at s0, 127/128 at s1).
"""

from contextlib import ExitStack

import numpy as np

N_PAIRS = 4
N_CORES = 8
H0, H1, H2 = 512, 256, 128
N_SCALES = 3
SMOOTH = 1e-5

# per-pair stats columns
C_CNT0A, C_CNT0B, C_CNT12 = 0, 1, 2
C_INT0, C_INT1, C_INT2 = 3, 4, 5
C_Z0, C_Z1, C_Z2 = 6, 7, 8
COLS_PER_PAIR = 10  # col 9 unused pad

# consts layout (fp8), free-dim offsets
W_IB2M = 0      # [2,128]  DR weights [I | B2M]
W_B2PI = 256    # [2,128]  [B2P | I]
W_II = 512      # [2,128]  [I | I]
W_ITRI = 768    # [2,128]  [I | TRI]
W_I = 1024      # [128]    plain identity
W_EVEN = 1152   # [64]     even-partition selector
N_CONST = 1216

_NC_CACHE = {}


def make_consts():
    import ml_dtypes

    ident = np.eye(128, dtype=np.float32)
    b2m = np.zeros((128, 128), np.float32)  # q in {i-1, i}
    b2p = np.zeros((128, 128), np.float32)  # q in {i, i+1}
    tri = np.zeros((128, 128), np.float32)  # q in {i-1, i, i+1}
    for i in range(128):
        for dq in (-1, 0):
            if 0 <= i + dq < 128:
                b2m[i + dq, i] = 1.0
        for dq in (0, 1):
            if 0 <= i + dq < 128:
                b2p[i + dq, i] = 1.0
        for dq in (-1, 0, 1):
            if 0 <= i + dq < 128:
                tri[i + dq, i] = 1.0
    even = np.zeros((128, 64), np.float32)
    for i in range(64):
        even[2 * i, i] = 1.0
    c = np.zeros((128, N_CONST), np.float32)
    c[:, 0:128], c[:, 128:256] = ident, b2m
    c[:, 256:384], c[:, 384:512] = b2p, ident
    c[:, 512:640], c[:, 640:768] = ident, ident
    c[:, 768:896], c[:, 896:1024] = ident, tri
    c[:, 1024:1152] = ident
    c[:, 1152:1216] = even
    return c.astype(ml_dtypes.float8_e4m3fn)


def build_kernel(n_pairs=N_PAIRS):
    import concourse.tile as tile
    from concourse import bacc, mybir
    from bass_rust import AP

    F32 = mybir.dt.float32
    F8 = mybir.dt.float8e4
    BF16 = mybir.dt.bfloat16
    I32 = mybir.dt.int32
    ALU = mybir.AluOpType
    ACTF = mybir.ActivationFunctionType
    DRM = mybir.MatmulPerfMode.DoubleRow

    ncols = n_pairs * COLS_PER_PAIR
    nc = bacc.Bacc("TRN2", target_bir_lowering=False, debug=False)

    logits0 = nc.dram_tensor("logits0", [n_pairs, H0, H0], F32, kind="ExternalInput").ap()
    logits1 = nc.dram_tensor("logits1", [n_pairs, H1, H1], F32, kind="ExternalInput").ap()
    logits2 = nc.dram_tensor("logits2", [n_pairs, H2, H2], F32, kind="ExternalInput").ap()
    targets = nc.dram_tensor("targets", [n_pairs, H0, H0], I32, kind="ExternalInput").ap()
    consts8 = nc.dram_tensor("consts_f8", [128, N_CONST], F8, kind="ExternalInput").ap()
    out = nc.dram_tensor("out", [128, ncols], F32, kind="ExternalOutput").ap()

    with tile.TileContext(nc) as tc, ExitStack() as ctx:
        singles = ctx.enter_context(tc.tile_pool(name="singles", bufs=1))
        tpool = ctx.enter_context(tc.tile_pool(name="tpool", bufs=4))
        lpool = ctx.enter_context(tc.tile_pool(name="lpool", bufs=4))
        ppool = ctx.enter_context(tc.tile_pool(name="ppool", bufs=4))
        spool = ctx.enter_context(tc.tile_pool(name="spool", bufs=2))
        ps0p = ctx.enter_context(tc.tile_pool(name="ps0p", bufs=2, space="PSUM"))
        ps12p = ctx.enter_context(tc.tile_pool(name="ps12p", bufs=2, space="PSUM"))

        cb = singles.tile([128, N_CONST], F8)
        nc.sync.dma_start(out=cb, in_=consts8)
        stats = singles.tile([128, ncols], F32)
        zeros1 = singles.tile([128, 1], BF16)
        nc.vector.memset(zeros1, 0.0)

        def wdr(off):
            return cb[:, off : off + 256].rearrange("p (two m) -> p two m", two=2)

        def windows(t, off, bstride, nb, istride, n):
            pstride = 1
            for s in t.tensor.shape[1:]:
                pstride *= s
            return AP(tensor=t.tensor, offset=off,
                      ap=[[pstride, 128], [bstride, nb], [istride, n]])

        # ---- all input DMAs queued up front ----
        # scalar queue: l1 + l2 (needed in later phases; issued before any
        # ACT compute so the engine is free once data starts landing)
        l1s, l2s = [], []
        for p in range(n_pairs):
            l1 = lpool.tile([128, 2, 256], F32, tag="l1", name=f"l1_{p}")
            nc.scalar.dma_start(
                out=l1, in_=logits1[p].rearrange("(g r) c -> r g c", g=2)
            )
            l1s.append(l1)
        for p in range(n_pairs):
            l2 = lpool.tile([128, 128], F32, tag="l2", name=f"l2_{p}")
            nc.scalar.dma_start(out=l2, in_=logits2[p])
            l2s.append(l2)
        # gpsimd queue: casting target DMAs
        t0s = []
        for p in range(n_pairs):
            t0 = tpool.tile([128, 2, 2, 516], F8, tag="t0", name=f"t0_{p}")
            for half in range(2):
                nc.gpsimd.dma_start(
                    out=t0[:, half, :, 2:514],
                    in_=targets[p, half * 256 : (half + 1) * 256].rearrange(
                        "(r parity) c -> r parity c", parity=2
                    ),
                )
            nc.gpsimd.memset(t0[:, :, :, 0:2], 0.0)
            nc.gpsimd.memset(t0[:, :, :, 514:516], 0.0)
            t0s.append(t0)
        # sync queue: logits0
        l0s = []
        for p in range(n_pairs):
            l0 = lpool.tile([128, 2, 2, 512], F32, tag="l0", name=f"l0_{p}")
            for half in range(2):
                nc.sync.dma_start(
                    out=l0[:, half],
                    in_=logits0[p, half * 256 : (half + 1) * 256].rearrange(
                        "(r parity) c -> r parity c", parity=2
                    ),
                )
            l0s.append(l0)

        stcol = lambda p, c: stats[:, p * COLS_PER_PAIR + c : p * COLS_PER_PAIR + c + 1]

        # ================= scale 0 =================
        # per pair: sigmoid+z (ACT), inter (DVE), conv (PE), counts (DVE)
        for p in range(n_pairs):
            t0, l0 = t0s[p], l0s[p]
            p0 = ppool.tile([128, 2, 2, 512], BF16, tag="p0", name=f"p0_{p}")
            nc.scalar.activation(out=p0, in_=l0, func=ACTF.Sigmoid)
            scr0 = spool.tile([128, 2, 2, 512], BF16, tag="scr0", name=f"scr0_{p}")
            nc.scalar.activation(out=scr0, in_=p0, func=ACTF.Square,
                                 accum_out=stcol(p, C_Z0))
            nc.vector.scalar_tensor_tensor(
                out=scr0, in0=p0, scalar=1.0, in1=t0[:, :, :, 2:514],
                op0=ALU.mult, op1=ALU.mult, accum_out=stcol(p, C_INT0),
            )
            for half in range(2):
                ps = ps0p.tile([128, 2, 512], F32, tag="ps0", name=f"ps0_{p}_{half}")
                for parity in range(2):
                    dst = ps[:, parity, :]
                    w1 = wdr(W_IB2M if parity == 0 else W_B2PI)
                    nc.tensor.matmul(dst, w1, t0[:, half, :, 2:514],
                                     start=True, stop=False, perf_mode=DRM)
                    off = (half * 2 + parity) * 516 + 1
                    nc.tensor.matmul(dst, wdr(W_II), windows(t0, off, 2, 2, 1, 512),
                                     start=False, stop=True, perf_mode=DRM)
                cnt0 = spool.tile([128, 2, 512], BF16, tag="cnt0", name=f"cnt0_{p}_{half}")
                nc.vector.scalar_tensor_tensor(
                    out=cnt0, in0=ps, scalar=-4.0,
                    in1=zeros1.broadcast_to([128, 2, 512]),
                    op0=ALU.add, op1=ALU.max,
                    accum_out=stcol(p, C_CNT0A + half),
                )

        # ================= scale 1 + scale 2 =================
        for p in range(n_pairs):
            t0, l1, l2 = t0s[p], l1s[p], l2s[p]
            # -- s1 --
            p1 = ppool.tile([128, 2, 256], BF16, tag="p1", name=f"p1_{p}")
            nc.scalar.activation(out=p1, in_=l1, func=ACTF.Sigmoid)
            scr1 = spool.tile([128, 2, 256], BF16, tag="scr1", name=f"scr1_{p}")
            nc.scalar.activation(out=scr1, in_=p1, func=ACTF.Square,
                                 accum_out=stcol(p, C_Z1))
            nc.vector.scalar_tensor_tensor(
                out=scr1, in0=p1, scalar=1.0, in1=t0[:, :, 0, 2:514:2],
                op0=ALU.mult, op1=ALU.mult, accum_out=stcol(p, C_INT1),
            )
            # ps12: [0:512]=s1 conv, [512:640]=s2 conv, [640:768]=t2 extract
            ps12 = ps12p.tile([128, 768], F32, tag="ps12", name=f"ps12_{p}")
            for g in range(2):
                dst = ps12[:, g * 256 : (g + 1) * 256]
                off = (g * 2) * 516
                nc.tensor.matmul(dst, wdr(W_ITRI), windows(t0, off, 2, 2, 2, 256),
                                 start=True, stop=False, perf_mode=DRM)
                nc.tensor.matmul(dst, cb[:, W_I : W_I + 128],
                                 t0[:, g, 0, 4:516:2], start=False, stop=True)
            # -- s2 --
            for g in range(2):
                nc.tensor.matmul(ps12[g * 64 : (g + 1) * 64, 640:768],
                                 cb[:, W_EVEN : W_EVEN + 64],
                                 t0[:, g, 0, 2:514:4], start=True, stop=True)
            t2 = tpool.tile([128, 130], F8, tag="t2", name=f"t2_{p}")
            nc.gpsimd.memset(t2[:, 0:1], 0.0)
            nc.gpsimd.memset(t2[:, 129:130], 0.0)
            nc.vector.tensor_copy(t2[:, 1:129], ps12[:, 640:768])

            p2 = ppool.tile([128, 128], BF16, tag="p2", name=f"p2_{p}")
            nc.scalar.activation(out=p2, in_=l2, func=ACTF.Sigmoid)
            scr2 = spool.tile([128, 128], BF16, tag="scr2", name=f"scr2_{p}")
            nc.scalar.activation(out=scr2, in_=p2, func=ACTF.Square,
                                 accum_out=stcol(p, C_Z2))
            nc.vector.scalar_tensor_tensor(
                out=scr2, in0=p2, scalar=1.0, in1=t2[:, 1:129],
                op0=ALU.mult, op1=ALU.mult, accum_out=stcol(p, C_INT2),
            )
            dst = ps12[:, 512:640]
            nc.tensor.matmul(dst, wdr(W_ITRI), windows(t2, 0, 1, 2, 1, 128),
                             start=True, stop=False, perf_mode=DRM)
            nc.tensor.matmul(dst, cb[:, W_I : W_I + 128], t2[:, 2:130],
                             start=False, stop=True)
            # merged s1+s2 count over contiguous [0:640]
            cnt12 = spool.tile([128, 640], BF16, tag="cnt12", name=f"cnt12_{p}")
            nc.vector.scalar_tensor_tensor(
                out=cnt12, in0=ps12[:, 0:640],
                scalar=-4.0, in1=zeros1.broadcast_to([128, 640]),
                op0=ALU.add, op1=ALU.max, accum_out=stcol(p, C_CNT12),
            )
            nc.sync.dma_start(
                out=out[:, p * COLS_PER_PAIR : p * COLS_PER_PAIR + 9],
                in_=stats[:, p * COLS_PER_PAIR : p * COLS_PER_PAIR + 9],
            )

    nc.compile()
    return nc


def get_kernel():
    if "nc" not in _NC_CACHE:
        _NC_CACHE["nc"] = build_kernel(N_PAIRS)
    return _NC_CACHE["nc"]


def seam_interior_counts(tg_pair):
    """Interior pixels in the seam rows the kernel cannot see (per scale)."""
    tg_pair = np.asarray(tg_pair)
    out = []
    for h in (H0, H1):
        step = H0 // h
        t = tg_pair[::step, ::step].astype(np.float64)
        pad = np.pad(t, 1)
        cnt = 0
        for r in (h // 2 - 1, h // 2):
            pr = r + 1
            nsum = (
                pad[pr, 1:-1] + pad[pr - 1, 1:-1] + pad[pr + 1, 1:-1]
                + pad[pr, 0:-2] + pad[pr, 2:]
            )
            cnt += int((nsum == 5.0).sum())
        out.append(float(cnt))
    out.append(0.0)
    return out


def combine_stats(all_core_outs, valid_mask, targets, n_pairs=N_PAIRS):
    vm = (np.asarray(valid_mask, np.float32).reshape(-1) >= 0.5).astype(np.float64)
    tg = np.asarray(targets).reshape(-1, H0, H0)
    n_total = vm.shape[0]
    per = np.zeros((N_SCALES, n_total), np.float64)
    for core, st in enumerate(all_core_outs):
        st = np.asarray(st, np.float64).sum(axis=0)  # reduce 128 partitions
        for j in range(n_pairs):
            g = core * n_pairs + j
            c = st[j * COLS_PER_PAIR : (j + 1) * COLS_PER_PAIR]
            seam = seam_interior_counts(tg[g])
            tgg = tg[g].astype(np.float64)
            host_S = [tgg.sum(), tgg[::2, ::2].sum(), tgg[::4, ::4].sum()]
            # cnt12 is the SUM of s1+s2 interiors; split using host-side
            # s2 interior count (cheap: 128x128).
            t2 = tgg[::4, ::4]
            pad = np.pad(t2, 1)
            nsum2 = (pad[1:-1, 1:-1] + pad[:-2, 1:-1] + pad[2:, 1:-1]
                     + pad[1:-1, :-2] + pad[1:-1, 2:])
            int2_host = float(((nsum2 == 5.0) & (t2 == 1.0)).sum())
            interior = [c[C_CNT0A] + c[C_CNT0B],
                        c[C_CNT12] - int2_host, int2_host]
            inter_v = [c[C_INT0], c[C_INT1], c[C_INT2]]
            z_v = [c[C_Z0], c[C_Z1], c[C_Z2]]
            for s in range(N_SCALES):
                S = host_S[s]
                C = S - (interior[s] + seam[s])
                alpha = min(2.0 * (1.0 - (C + SMOOTH) / (S + SMOOTH)) - 1.0, 0.8)
                dou = (z_v[s] + S - 2.0 * inter_v[s] + SMOOTH) / (
                    z_v[s] + S - (1.0 + alpha) * inter_v[s] + SMOOTH
                )
                per[s, g] = dou if S > 0 else 0.0
    cnt = vm.sum()
    ws = np.array([1.0, 0.5, 0.25])
    ws = ws / ws.sum()
    loss = 0.0
    for s in range(N_SCALES):
        ls = (per[s] * vm).sum() / cnt if cnt > 0 else 0.0
        loss += ws[s] * ls
    return np.float32(loss)


def make_in_maps(inputs):
    l0 = np.ascontiguousarray(np.asarray(inputs["logits0"], np.float32).reshape(-1, H0, H0))
    l1 = np.ascontiguousarray(np.asarray(inputs["logits1"], np.float32).reshape(-1, H1, H1))
    l2 = np.ascontiguousarray(np.asarray(inputs["logits2"], np.float32).reshape(-1, H2, H2))
    tg = np.ascontiguousarray(np.asarray(inputs["targets"], np.int32).reshape(-1, H0, H0))
    consts = np.asarray(make_consts())
    in_maps = []
    for core in range(N_CORES):
        lo, hi = core * N_PAIRS, (core + 1) * N_PAIRS
        in_maps.append({
            "logits0": np.ascontiguousarray(l0[lo:hi]),
            "logits1": np.ascontiguousarray(l1[lo:hi]),
            "logits2": np.ascontiguousarray(l2[lo:hi]),
            "targets": np.ascontiguousarray(tg[lo:hi]),
            "consts_f8": consts,
        })
    return in_maps


def run_cores(inputs, **spmd_kwargs):
    from concourse.bass_utils import run_bass_kernel_spmd

    nc = get_kernel()
    in_maps = make_in_maps(inputs)
    return run_bass_kernel_spmd(nc, in_maps, core_ids=list(range(N_CORES)), **spmd_kwargs)


def kernel(**inputs) -> np.ndarray:
    res = run_cores(inputs)
    outs = [res.results[c]["out"] for c in range(N_CORES)]
    return combine_stats(outs, inputs["valid_mask"], inputs["targets"])


# revision 8
# speedup vs baseline: 1.1698x; 1.1336x over previous
"""Trainium2 Bass kernel for nn_DeepSupervisionBoundaryDoULoss.

kernel(**inputs) takes the FULL unsharded inputs (logits0/1/2, targets,
valid_mask) and returns the full scalar loss (float32).

Strategy: data-parallel over the 32 (b,n) pairs -> 4 pairs per core x 8 cores.
Each core streams its slice (~9.25 MiB) once; DMA is the roofline (~29us at
~340 GB/s/core). Pair-major pipeline; every queue carries the data in the
order compute consumes it (targets on the gpsimd SWDGE queue, logits0 on the
sync HWDGE queue, logits1+2 on the scalar HWDGE queue).

  - Pool(gpsimd): int32->fp8e4 casting DMAs for targets (deinterleaved rows
    A/B/C/D as [128, half, parity, 516] w/ 2-col zero pads), pad memsets.
  - ACT(scalar):  logits1/2 DMA issue (in the dead preamble window); sigmoid
    (f32->bf16, one op for merged s1+s2); z = sum(p^2) via Square w/ accum.
  - DVE(vector):  inter = sum(p*t) via scalar_tensor_tensor (bf16 x fp8);
    interior counts via stt-relu ((nsum-4) max 0) straight from PSUM with
    accum_out (one op over the 4-bank s0 PSUM + one merged s1+s2 op);
    t2 extraction copy.
  - PE(tensor):   3x3-cross conv as fp8 DoubleRow matmuls: per tile ONE DR
    fuses [I|band]@[center|other-parity] and ONE DR fuses [I|I]@[left|right]
    (overlapping strided ifmap APs), accumulated in PSUM f32.
  - Sync(SP):     consts + logits0 DMAs + per-pair stats out-DMAs.

All engine scratch is per-op-tagged so ACT and DVE never couple through
buffer reuse. Stats land in [128,10] f32 blocks per pair (partition-reduced
on host); the host finishes alpha/dou/weighted mean plus a seam correction
for the 4 rows/pair the on-chip conv cannot see (rows 255/256 at s0,
127/128 at s1), and splits the merged s1+s2 count using the (targets-only)
host s2 interior count.
"""

from contextlib import ExitStack

import numpy as np

N_PAIRS = 4
N_CORES = 8
H0, H1, H2 = 512, 256, 128
N_SCALES = 3
SMOOTH = 1e-5

# per-pair stats columns
C_CNT0, C_CNT12 = 0, 2
C_INT0, C_INT1, C_INT2 = 3, 4, 5
C_Z0, C_Z1, C_Z2 = 6, 7, 8
COLS_PER_PAIR = 10

# consts layout (fp8), free-dim offsets
W_IB2M = 0      # [2,128]  DR weights [I | B2M]
W_B2PI = 256    # [2,128]  [B2P | I]
W_II = 512      # [2,128]  [I | I]
W_ITRI = 768    # [2,128]  [I | TRI]
W_I = 1024      # [128]    plain identity
W_EVEN = 1152   # [64]     even-partition selector
N_CONST = 1216

_NC_CACHE = {}


def make_consts():
    import ml_dtypes

    ident = np.eye(128, dtype=np.float32)
    b2m = np.zeros((128, 128), np.float32)  # q in {i-1, i}
    b2p = np.zeros((128, 128), np.float32)  # q in {i, i+1}
    tri = np.zeros((128, 128), np.float32)  # q in {i-1, i, i+1}
    for i in range(128):
        for dq in (-1, 0):
            if 0 <= i + dq < 128:
                b2m[i + dq, i] = 1.0
        for dq in (0, 1):
            if 0 <= i + dq < 128:
                b2p[i + dq, i] = 1.0
        for dq in (-1, 0, 1):
            if 0 <= i + dq < 128:
                tri[i + dq, i] = 1.0
    even = np.zeros((128, 64), np.float32)
    for i in range(64):
        even[2 * i, i] = 1.0
    c = np.zeros((128, N_CONST), np.float32)
    c[:, 0:128], c[:, 128:256] = ident, b2m
    c[:, 256:384], c[:, 384:512] = b2p, ident
    c[:, 512:640], c[:, 640:768] = ident, ident
    c[:, 768:896], c[:, 896:1024] = ident, tri
    c[:, 1024:1152] = ident
    c[:, 1152:1216] = even
    return c.astype(ml_dtypes.float8_e4m3fn)


def build_kernel(n_pairs=N_PAIRS):
    import concourse.tile as tile
    from concourse import bacc, mybir
    from bass_rust import AP

    F32 = mybir.dt.float32
    F8 = mybir.dt.float8e4
    BF16 = mybir.dt.bfloat16
    I32 = mybir.dt.int32
    ALU = mybir.AluOpType
    ACTF = mybir.ActivationFunctionType
    DRM = mybir.MatmulPerfMode.DoubleRow

    ncols = n_pairs * COLS_PER_PAIR
    nc = bacc.Bacc("TRN2", target_bir_lowering=False, debug=False)

    logits0 = nc.dram_tensor("logits0", [n_pairs, H0, H0], F32, kind="ExternalInput").ap()
    logits1 = nc.dram_tensor("logits1", [n_pairs, H1, H1], F32, kind="ExternalInput").ap()
    logits2 = nc.dram_tensor("logits2", [n_pairs, H2, H2], F32, kind="ExternalInput").ap()
    targets = nc.dram_tensor("targets", [n_pairs, H0, H0], I32, kind="ExternalInput").ap()
    consts8 = nc.dram_tensor("consts_f8", [128, N_CONST], F8, kind="ExternalInput").ap()
    out = nc.dram_tensor("out", [128, ncols], F32, kind="ExternalOutput").ap()

    with tile.TileContext(nc) as tc, ExitStack() as ctx:
        singles = ctx.enter_context(tc.tile_pool(name="singles", bufs=1))
        tpool = ctx.enter_context(tc.tile_pool(name="tpool", bufs=4))
        lpool = ctx.enter_context(tc.tile_pool(name="lpool", bufs=4))
        ppool = ctx.enter_context(tc.tile_pool(name="ppool", bufs=4))
        spool = ctx.enter_context(tc.tile_pool(name="spool", bufs=2))
        ps0p = ctx.enter_context(tc.tile_pool(name="ps0p", bufs=1, space="PSUM"))
        ps12p = ctx.enter_context(tc.tile_pool(name="ps12p", bufs=2, space="PSUM"))

        cb = singles.tile([128, N_CONST], F8)
        nc.sync.dma_start(out=cb, in_=consts8)
        stats = singles.tile([128, ncols], F32)
        nc.vector.memset(stats, 0.0)
        zeros1 = singles.tile([128, 1], BF16)
        nc.vector.memset(zeros1, 0.0)

        def wdr(off):
            return cb[:, off : off + 256].rearrange("p (two m) -> p two m", two=2)

        def windows(t, off, bstride, nb, istride, n):
            pstride = 1
            for s in t.tensor.shape[1:]:
                pstride *= s
            return AP(tensor=t.tensor, offset=off,
                      ap=[[pstride, 128], [bstride, nb], [istride, n]])

        # ---- all input DMAs queued up front, per-queue in consumption order
        l12s = []
        for p in range(n_pairs):
            l12 = lpool.tile([128, 640], F32, tag="l12", name=f"l12_{p}")
            nc.scalar.dma_start(
                out=l12[:, 0:512].rearrange("r (g c) -> r g c", g=2),
                in_=logits1[p].rearrange("(g r) c -> r g c", g=2),
            )
            nc.scalar.dma_start(out=l12[:, 512:640], in_=logits2[p])
            l12s.append(l12)
        t0s = []
        for p in range(n_pairs):
            t0 = tpool.tile([128, 2, 2, 516], F8, tag="t0", name=f"t0_{p}")
            for half in range(2):
                nc.gpsimd.dma_start(
                    out=t0[:, half, :, 2:514],
                    in_=targets[p, half * 256 : (half + 1) * 256].rearrange(
                        "(r parity) c -> r parity c", parity=2
                    ),
                )
            nc.gpsimd.memset(t0[:, :, :, 0:2], 0.0)
            nc.gpsimd.memset(t0[:, :, :, 514:516], 0.0)
            t0s.append(t0)
        l0s = []
        for p in range(n_pairs):
            l0 = lpool.tile([128, 2, 2, 512], F32, tag="l0", name=f"l0_{p}")
            for half in range(2):
                nc.sync.dma_start(
                    out=l0[:, half],
                    in_=logits0[p, half * 256 : (half + 1) * 256].rearrange(
                        "(r parity) c -> r parity c", parity=2
                    ),
                )
            l0s.append(l0)

        stcol = lambda p, c: stats[:, p * COLS_PER_PAIR + c : p * COLS_PER_PAIR + c + 1]

        for p in range(n_pairs):
            t0, l0, l12 = t0s[p], l0s[p], l12s[p]

            # ---------------- scale 0 ----------------
            p0 = ppool.tile([128, 2, 2, 512], BF16, tag="p0", name=f"p0_{p}")
            nc.scalar.activation(out=p0, in_=l0, func=ACTF.Sigmoid)
            sqr0 = spool.tile([128, 2, 2, 512], BF16, tag="sqr0", name=f"sqr0_{p}")
            nc.scalar.activation(out=sqr0, in_=p0, func=ACTF.Square,
                                 accum_out=stcol(p, C_Z0))
            int0 = spool.tile([128, 2, 2, 512], BF16, tag="int0", name=f"int0_{p}")
            nc.vector.scalar_tensor_tensor(
                out=int0, in0=p0, scalar=1.0, in1=t0[:, :, :, 2:514],
                op0=ALU.mult, op1=ALU.mult, accum_out=stcol(p, C_INT0),
            )
            ps0 = ps0p.tile([128, 4, 512], F32, tag="ps0", name=f"ps0_{p}")
            for half in range(2):
                for parity in range(2):
                    dst = ps0[:, half * 2 + parity, :]
                    w1 = wdr(W_IB2M if parity == 0 else W_B2PI)
                    nc.tensor.matmul(dst, w1, t0[:, half, :, 2:514],
                                     start=True, stop=False, perf_mode=DRM)
                    off = (half * 2 + parity) * 516 + 1
                    nc.tensor.matmul(dst, wdr(W_II), windows(t0, off, 2, 2, 1, 512),
                                     start=False, stop=True, perf_mode=DRM)
            cnt0 = spool.tile([128, 4, 512], BF16, tag="cnt0", name=f"cnt0_{p}")
            nc.vector.scalar_tensor_tensor(
                out=cnt0, in0=ps0, scalar=-4.0,
                in1=zeros1.broadcast_to([128, 4, 512]),
                op0=ALU.add, op1=ALU.max, accum_out=stcol(p, C_CNT0),
            )

            # ---------------- scales 1+2 ----------------
            p12 = ppool.tile([128, 640], BF16, tag="p12", name=f"p12_{p}")
            nc.scalar.activation(out=p12, in_=l12, func=ACTF.Sigmoid)
            nc.scalar.activation(
                out=spool.tile([128, 512], BF16, tag="sqr1", name=f"sqr1_{p}"),
                in_=p12[:, 0:512], func=ACTF.Square, accum_out=stcol(p, C_Z1))
            nc.scalar.activation(
                out=spool.tile([128, 128], BF16, tag="sqr2", name=f"sqr2_{p}"),
                in_=p12[:, 512:640], func=ACTF.Square, accum_out=stcol(p, C_Z2))
            int1 = spool.tile([128, 2, 256], BF16, tag="int1", name=f"int1_{p}")
            nc.vector.scalar_tensor_tensor(
                out=int1, in0=p12[:, 0:512].rearrange("r (g c) -> r g c", g=2),
                scalar=1.0, in1=t0[:, :, 0, 2:514:2],
                op0=ALU.mult, op1=ALU.mult, accum_out=stcol(p, C_INT1),
            )
            # ps12: [0:512]=s1 conv, [512:640]=s2 conv, [640:768]=t2 extract
            ps12 = ps12p.tile([128, 768], F32, tag="ps12", name=f"ps12_{p}")
            for g in range(2):
                dst = ps12[:, g * 256 : (g + 1) * 256]
                off = (g * 2) * 516
                nc.tensor.matmul(dst, wdr(W_ITRI), windows(t0, off, 2, 2, 2, 256),
                                 start=True, stop=False, perf_mode=DRM)
                nc.tensor.matmul(dst, cb[:, W_I : W_I + 128],
                                 t0[:, g, 0, 4:516:2], start=False, stop=True)
            for g in range(2):
                nc.tensor.matmul(ps12[g * 64 : (g + 1) * 64, 640:768],
                                 cb[:, W_EVEN : W_EVEN + 64],
                                 t0[:, g, 0, 2:514:4], start=True, stop=True)
            t2 = tpool.tile([128, 130], F8, tag="t2", name=f"t2_{p}")
            nc.gpsimd.memset(t2[:, 0:1], 0.0)
            nc.gpsimd.memset(t2[:, 129:130], 0.0)
            nc.vector.tensor_copy(t2[:, 1:129], ps12[:, 640:768])
            int2 = spool.tile([128, 128], BF16, tag="int2", name=f"int2_{p}")
            nc.vector.scalar_tensor_tensor(
                out=int2, in0=p12[:, 512:640], scalar=1.0, in1=t2[:, 1:129],
                op0=ALU.mult, op1=ALU.mult, accum_out=stcol(p, C_INT2),
            )
            dst = ps12[:, 512:640]
            nc.tensor.matmul(dst, wdr(W_ITRI), windows(t2, 0, 1, 2, 1, 128),
                             start=True, stop=False, perf_mode=DRM)
            nc.tensor.matmul(dst, cb[:, W_I : W_I + 128], t2[:, 2:130],
                             start=False, stop=True)
            cnt12 = spool.tile([128, 640], BF16, tag="cnt12", name=f"cnt12_{p}")
            nc.vector.scalar_tensor_tensor(
                out=cnt12, in0=ps12[:, 0:640],
                scalar=-4.0, in1=zeros1.broadcast_to([128, 640]),
                op0=ALU.add, op1=ALU.max, accum_out=stcol(p, C_CNT12),
            )
            nc.sync.dma_start(
                out=out[:, p * COLS_PER_PAIR : p * COLS_PER_PAIR + 9],
                in_=stats[:, p * COLS_PER_PAIR : p * COLS_PER_PAIR + 9],
            )

    nc.compile()
    return nc


def get_kernel():
    if "nc" not in _NC_CACHE:
        _NC_CACHE["nc"] = build_kernel(N_PAIRS)
    return _NC_CACHE["nc"]


def seam_interior_counts(tg_pair):
    """Interior pixels in the seam rows the kernel cannot see (per scale)."""
    tg_pair = np.asarray(tg_pair)
    out = []
    for h in (H0, H1):
        step = H0 // h
        t = tg_pair[::step, ::step].astype(np.float64)
        pad = np.pad(t, 1)
        cnt = 0
        for r in (h // 2 - 1, h // 2):
            pr = r + 1
            nsum = (
                pad[pr, 1:-1] + pad[pr - 1, 1:-1] + pad[pr + 1, 1:-1]
                + pad[pr, 0:-2] + pad[pr, 2:]
            )
            cnt += int((nsum == 5.0).sum())
        out.append(float(cnt))
    out.append(0.0)
    return out


def combine_stats(all_core_outs, valid_mask, targets, n_pairs=N_PAIRS):
    vm = (np.asarray(valid_mask, np.float32).reshape(-1) >= 0.5).astype(np.float64)
    tg = np.asarray(targets).reshape(-1, H0, H0)
    n_total = vm.shape[0]
    per = np.zeros((N_SCALES, n_total), np.float64)
    for core, st in enumerate(all_core_outs):
        st = np.asarray(st, np.float64).sum(axis=0)  # reduce 128 partitions
        for j in range(n_pairs):
            g = core * n_pairs + j
            c = st[j * COLS_PER_PAIR : (j + 1) * COLS_PER_PAIR]
            seam = seam_interior_counts(tg[g])
            tgg = tg[g].astype(np.float64)
            host_S = [tgg.sum(), tgg[::2, ::2].sum(), tgg[::4, ::4].sum()]
            # cnt12 is the s1+s2 interior sum; split via host s2 count
            t2 = tgg[::4, ::4]
            pad = np.pad(t2, 1)
            nsum2 = (pad[1:-1, 1:-1] + pad[:-2, 1:-1] + pad[2:, 1:-1]
                     + pad[1:-1, :-2] + pad[1:-1, 2:])
            int2_host = float(((nsum2 == 5.0) & (t2 == 1.0)).sum())
            interior = [c[C_CNT0], c[C_CNT12] - int2_host, int2_host]
            inter_v = [c[C_INT0], c[C_INT1], c[C_INT2]]
            z_v = [c[C_Z0], c[C_Z1], c[C_Z2]]
            for s in range(N_SCALES):
                S = host_S[s]
                C = S - (interior[s] + seam[s])
                alpha = min(2.0 * (1.0 - (C + SMOOTH) / (S + SMOOTH)) - 1.0, 0.8)
                dou = (z_v[s] + S - 2.0 * inter_v[s] + SMOOTH) / (
                    z_v[s] + S - (1.0 + alpha) * inter_v[s] + SMOOTH
                )
                per[s, g] = dou if S > 0 else 0.0
    cnt = vm.sum()
    ws = np.array([1.0, 0.5, 0.25])
    ws = ws / ws.sum()
    loss = 0.0
    for s in range(N_SCALES):
        ls = (per[s] * vm).sum() / cnt if cnt > 0 else 0.0
        loss += ws[s] * ls
    return np.float32(loss)


def make_in_maps(inputs):
    l0 = np.ascontiguousarray(np.asarray(inputs["logits0"], np.float32).reshape(-1, H0, H0))
    l1 = np.ascontiguousarray(np.asarray(inputs["logits1"], np.float32).reshape(-1, H1, H1))
    l2 = np.ascontiguousarray(np.asarray(inputs["logits2"], np.float32).reshape(-1, H2, H2))
    tg = np.ascontiguousarray(np.asarray(inputs["targets"], np.int32).reshape(-1, H0, H0))
    consts = np.asarray(make_consts())
    in_maps = []
    for core in range(N_CORES):
        lo, hi = core * N_PAIRS, (core + 1) * N_PAIRS
        in_maps.append({
            "logits0": np.ascontiguousarray(l0[lo:hi]),
            "logits1": np.ascontiguousarray(l1[lo:hi]),
            "logits2": np.ascontiguousarray(l2[lo:hi]),
            "targets": np.ascontiguousarray(tg[lo:hi]),
            "consts_f8": consts,
        })
    return in_maps


def run_cores(inputs, **spmd_kwargs):
    from concourse.bass_utils import run_bass_kernel_spmd

    nc = get_kernel()
    in_maps = make_in_maps(inputs)
    return run_bass_kernel_spmd(nc, in_maps, core_ids=list(range(N_CORES)), **spmd_kwargs)


def kernel(**inputs) -> np.ndarray:
    res = run_cores(inputs)
    outs = [res.results[c]["out"] for c in range(N_CORES)]
    return combine_stats(outs, inputs["valid_mask"], inputs["targets"])


# revision 13
# speedup vs baseline: 1.1884x; 1.0159x over previous
"""Trainium2 Bass kernel for nn_DeepSupervisionBoundaryDoULoss.

kernel(**inputs) takes the FULL unsharded inputs (logits0/1/2, targets,
valid_mask) and returns the full scalar loss (float32).

Strategy: data-parallel over the 32 (b,n) pairs -> 4 pairs per core x 8 cores.
Each core streams its slice (~9.25 MiB) once; DMA is the roofline (~29us at
~340 GB/s/core). Pair-major pipeline; every queue carries the data in the
order compute consumes it (targets on the gpsimd SWDGE queue, logits0 on the
sync HWDGE queue, logits1+2 on the scalar HWDGE queue).

  - Pool(gpsimd): int32->fp8e4 casting DMAs for targets (deinterleaved rows
    A/B/C/D as [128, half, parity, 516] w/ 2-col zero pads), pad memsets.
  - ACT(scalar):  logits1/2 DMA issue (in the dead preamble window); sigmoid
    (f32->bf16, one op for merged s1+s2); z = sum(p^2) via Square w/ accum.
  - DVE(vector):  inter = sum(p*t) via scalar_tensor_tensor (bf16 x fp8);
    interior counts via stt-relu ((nsum-4) max 0) straight from PSUM with
    accum_out (one op over the 4-bank s0 PSUM + one merged s1+s2 op);
    t2 extraction copy.
  - PE(tensor):   3x3-cross conv as fp8 DoubleRow matmuls: per tile ONE DR
    fuses [I|band]@[center|other-parity] and ONE DR fuses [I|I]@[left|right]
    (overlapping strided ifmap APs), accumulated in PSUM f32.
  - Sync(SP):     consts + logits0 DMAs + per-pair stats out-DMAs.

All engine scratch is per-op-tagged so ACT and DVE never couple through
buffer reuse. Stats land in [128,10] f32 blocks per pair (partition-reduced
on host); the host finishes alpha/dou/weighted mean plus a seam correction
for the 4 rows/pair the on-chip conv cannot see (rows 255/256 at s0,
127/128 at s1), and splits the merged s1+s2 count using the (targets-only)
host s2 interior count.
"""

from contextlib import ExitStack

import numpy as np

N_PAIRS = 4
N_CORES = 8
H0, H1, H2 = 512, 256, 128
N_SCALES = 3
SMOOTH = 1e-5

# per-pair stats columns (s0 quantities split per half)
C_CNT0A, C_CNT0B, C_CNT1 = 0, 1, 2
C_INT0A, C_INT0B, C_INT1, C_INT2 = 3, 4, 5, 6
C_Z0A, C_Z0B, C_Z1, C_Z2 = 7, 8, 9, 10
COLS_PER_PAIR = 12

# consts layout (fp8), free-dim offsets
W_IB2M = 0      # [2,128]  DR weights [I | B2M]
W_B2PI = 256    # [2,128]  [B2P | I]
W_II = 512      # [2,128]  [I | I]
W_ITRI = 768    # [2,128]  [I | TRI]
W_I = 1024      # [128]    plain identity
W_EVEN = 1152   # [64]     even-partition selector
N_CONST = 1216

_NC_CACHE = {}


def make_consts():
    import ml_dtypes

    ident = np.eye(128, dtype=np.float32)
    b2m = np.zeros((128, 128), np.float32)  # q in {i-1, i}
    b2p = np.zeros((128, 128), np.float32)  # q in {i, i+1}
    tri = np.zeros((128, 128), np.float32)  # q in {i-1, i, i+1}
    for i in range(128):
        for dq in (-1, 0):
            if 0 <= i + dq < 128:
                b2m[i + dq, i] = 1.0
        for dq in (0, 1):
            if 0 <= i + dq < 128:
                b2p[i + dq, i] = 1.0
        for dq in (-1, 0, 1):
            if 0 <= i + dq < 128:
                tri[i + dq, i] = 1.0
    even = np.zeros((128, 64), np.float32)
    for i in range(64):
        even[2 * i, i] = 1.0
    c = np.zeros((128, N_CONST), np.float32)
    c[:, 0:128], c[:, 128:256] = ident, b2m
    c[:, 256:384], c[:, 384:512] = b2p, ident
    c[:, 512:640], c[:, 640:768] = ident, ident
    c[:, 768:896], c[:, 896:1024] = ident, tri
    c[:, 1024:1152] = ident
    c[:, 1152:1216] = even
    return c.astype(ml_dtypes.float8_e4m3fn)


def build_kernel(n_pairs=N_PAIRS):
    import concourse.tile as tile
    from concourse import bacc, mybir
    from bass_rust import AP

    F32 = mybir.dt.float32
    F8 = mybir.dt.float8e4
    BF16 = mybir.dt.bfloat16
    I32 = mybir.dt.int32
    ALU = mybir.AluOpType
    ACTF = mybir.ActivationFunctionType
    DRM = mybir.MatmulPerfMode.DoubleRow

    ncols = n_pairs * COLS_PER_PAIR
    nc = bacc.Bacc("TRN2", target_bir_lowering=False, debug=False)

    logits0 = nc.dram_tensor("logits0", [n_pairs, H0, H0], F32, kind="ExternalInput").ap()
    logits1 = nc.dram_tensor("logits1", [n_pairs, H1, H1], F32, kind="ExternalInput").ap()
    logits2 = nc.dram_tensor("logits2", [n_pairs, H2, H2], F32, kind="ExternalInput").ap()
    targets = nc.dram_tensor("targets", [n_pairs, H0, H0], I32, kind="ExternalInput").ap()
    consts8 = nc.dram_tensor("consts_f8", [128, N_CONST], F8, kind="ExternalInput").ap()
    out = nc.dram_tensor("out", [128, ncols], F32, kind="ExternalOutput").ap()

    with tile.TileContext(nc) as tc, ExitStack() as ctx:
        singles = ctx.enter_context(tc.tile_pool(name="singles", bufs=1))
        tpool = ctx.enter_context(tc.tile_pool(name="tpool", bufs=4))
        lpool = ctx.enter_context(tc.tile_pool(name="lpool", bufs=4))
        ppool = ctx.enter_context(tc.tile_pool(name="ppool", bufs=4))
        spool = ctx.enter_context(tc.tile_pool(name="spool", bufs=2))
        ps0p = ctx.enter_context(tc.tile_pool(name="ps0p", bufs=2, space="PSUM"))
        ps12p = ctx.enter_context(tc.tile_pool(name="ps12p", bufs=2, space="PSUM"))

        cb = singles.tile([128, N_CONST], F8)
        nc.sync.dma_start(out=cb, in_=consts8)
        stats = singles.tile([128, ncols], F32)
        nc.vector.memset(stats, 0.0)
        zeros1 = singles.tile([128, 1], BF16)
        nc.vector.memset(zeros1, 0.0)

        def wdr(off):
            return cb[:, off : off + 256].rearrange("p (two m) -> p two m", two=2)

        def windows(t, off, bstride, nb, istride, n):
            pstride = 1
            for s in t.tensor.shape[1:]:
                pstride *= s
            return AP(tensor=t.tensor, offset=off,
                      ap=[[pstride, 128], [bstride, nb], [istride, n]])

        # ---- all input DMAs queued up front, per-queue in consumption order
        l12s = []
        for p in range(n_pairs):
            l12 = lpool.tile([128, 640], F32, tag="l12", name=f"l12_{p}")
            nc.scalar.dma_start(
                out=l12[:, 0:512].rearrange("r (g c) -> r g c", g=2),
                in_=logits1[p].rearrange("(g r) c -> r g c", g=2),
            )
            nc.scalar.dma_start(out=l12[:, 512:640], in_=logits2[p])
            l12s.append(l12)
        t0s = []
        for p in range(n_pairs):
            t0 = tpool.tile([128, 2, 2, 516], F8, tag="t0", name=f"t0_{p}")
            for half in range(2):
                nc.gpsimd.dma_start(
                    out=t0[:, half, :, 2:514],
                    in_=targets[p, half * 256 : (half + 1) * 256].rearrange(
                        "(r parity) c -> r parity c", parity=2
                    ),
                )
            nc.gpsimd.memset(t0[:, :, :, 0:2], 0.0)
            nc.gpsimd.memset(t0[:, :, :, 514:516], 0.0)
            t0s.append(t0)
        l0s = []
        for p in range(n_pairs):
            l0 = lpool.tile([128, 2, 2, 512], F32, tag="l0", name=f"l0_{p}")
            for half in range(2):
                nc.sync.dma_start(
                    out=l0[:, half],
                    in_=logits0[p, half * 256 : (half + 1) * 256].rearrange(
                        "(r parity) c -> r parity c", parity=2
                    ),
                )
            l0s.append(l0)

        stcol = lambda p, c: stats[:, p * COLS_PER_PAIR + c : p * COLS_PER_PAIR + c + 1]

        for p in range(n_pairs):
            t0, l0, l12 = t0s[p], l0s[p], l12s[p]

            # ---------------- scale 0 (per-half pipeline) ----------------
            p0 = ppool.tile([128, 2, 2, 512], BF16, tag="p0", name=f"p0_{p}")
            for half in range(2):
                nc.scalar.activation(out=p0[:, half], in_=l0[:, half],
                                     func=ACTF.Sigmoid)
                sqr0 = spool.tile([128, 2, 512], BF16, tag=f"sqr0{half}",
                                  name=f"sqr0_{p}_{half}")
                nc.scalar.activation(out=sqr0, in_=p0[:, half], func=ACTF.Square,
                                     accum_out=stcol(p, C_Z0A + half))
                int0 = spool.tile([128, 2, 512], BF16, tag=f"int0{half}",
                                  name=f"int0_{p}_{half}")
                nc.vector.scalar_tensor_tensor(
                    out=int0, in0=p0[:, half], scalar=1.0,
                    in1=t0[:, half, :, 2:514],
                    op0=ALU.mult, op1=ALU.mult,
                    accum_out=stcol(p, C_INT0A + half),
                )
                ps0 = ps0p.tile([128, 2, 512], F32, tag="ps0",
                                name=f"ps0_{p}_{half}")
                for parity in range(2):
                    dst = ps0[:, parity, :]
                    w1 = wdr(W_IB2M if parity == 0 else W_B2PI)
                    nc.tensor.matmul(dst, w1, t0[:, half, :, 2:514],
                                     start=True, stop=False, perf_mode=DRM)
                    off = (half * 2 + parity) * 516 + 1
                    nc.tensor.matmul(dst, wdr(W_II), windows(t0, off, 2, 2, 1, 512),
                                     start=False, stop=True, perf_mode=DRM)
                cnt0 = spool.tile([128, 2, 512], BF16, tag=f"cnt0{half}",
                                  name=f"cnt0_{p}_{half}")
                nc.vector.scalar_tensor_tensor(
                    out=cnt0, in0=ps0, scalar=-4.0,
                    in1=zeros1.broadcast_to([128, 2, 512]),
                    op0=ALU.add, op1=ALU.max, accum_out=stcol(p, C_CNT0A + half),
                )

            # ---------------- scales 1+2 ----------------
            p12 = ppool.tile([128, 640], BF16, tag="p12", name=f"p12_{p}")
            nc.scalar.activation(out=p12, in_=l12, func=ACTF.Sigmoid)
            nc.scalar.activation(
                out=spool.tile([128, 512], BF16, tag="sqr1", name=f"sqr1_{p}"),
                in_=p12[:, 0:512], func=ACTF.Square, accum_out=stcol(p, C_Z1))
            nc.scalar.activation(
                out=spool.tile([128, 128], BF16, tag="sqr2", name=f"sqr2_{p}"),
                in_=p12[:, 512:640], func=ACTF.Square, accum_out=stcol(p, C_Z2))
            int1 = spool.tile([128, 2, 256], BF16, tag="int1", name=f"int1_{p}")
            nc.vector.scalar_tensor_tensor(
                out=int1, in0=p12[:, 0:512].rearrange("r (g c) -> r g c", g=2),
                scalar=1.0, in1=t0[:, :, 0, 2:514:2],
                op0=ALU.mult, op1=ALU.mult, accum_out=stcol(p, C_INT1),
            )
            # ps12: [0:512]=s1 conv, [512:640]=t2 extract. (The s2 interior
            # count is recovered on the host from targets, which it needs
            # anyway -- no on-device s2 conv.)
            ps12 = ps12p.tile([128, 640], F32, tag="ps12", name=f"ps12_{p}")
            for g in range(2):
                dst = ps12[:, g * 256 : (g + 1) * 256]
                off = (g * 2) * 516
                nc.tensor.matmul(dst, wdr(W_ITRI), windows(t0, off, 2, 2, 2, 256),
                                 start=True, stop=False, perf_mode=DRM)
                nc.tensor.matmul(dst, cb[:, W_I : W_I + 128],
                                 t0[:, g, 0, 4:516:2], start=False, stop=True)
            for g in range(2):
                nc.tensor.matmul(ps12[g * 64 : (g + 1) * 64, 512:640],
                                 cb[:, W_EVEN : W_EVEN + 64],
                                 t0[:, g, 0, 2:514:4], start=True, stop=True)
            t2 = tpool.tile([128, 128], F8, tag="t2", name=f"t2_{p}")
            nc.vector.tensor_copy(t2, ps12[:, 512:640])
            int2 = spool.tile([128, 128], BF16, tag="int2", name=f"int2_{p}")
            nc.vector.scalar_tensor_tensor(
                out=int2, in0=p12[:, 512:640], scalar=1.0, in1=t2,
                op0=ALU.mult, op1=ALU.mult, accum_out=stcol(p, C_INT2),
            )
            cnt1 = spool.tile([128, 512], BF16, tag="cnt1", name=f"cnt1_{p}")
            nc.vector.scalar_tensor_tensor(
                out=cnt1, in0=ps12[:, 0:512],
                scalar=-4.0, in1=zeros1.broadcast_to([128, 512]),
                op0=ALU.add, op1=ALU.max, accum_out=stcol(p, C_CNT1),
            )
            nc.sync.dma_start(
                out=out[:, p * COLS_PER_PAIR : p * COLS_PER_PAIR + 11],
                in_=stats[:, p * COLS_PER_PAIR : p * COLS_PER_PAIR + 11],
            )

    nc.compile()
    return nc


def get_kernel():
    if "nc" not in _NC_CACHE:
        _NC_CACHE["nc"] = build_kernel(N_PAIRS)
    return _NC_CACHE["nc"]


def seam_interior_counts(tg_pair):
    """Interior pixels in the seam rows the kernel cannot see (per scale)."""
    tg_pair = np.asarray(tg_pair)
    out = []
    for h in (H0, H1):
        step = H0 // h
        t = tg_pair[::step, ::step].astype(np.float64)
        pad = np.pad(t, 1)
        cnt = 0
        for r in (h // 2 - 1, h // 2):
            pr = r + 1
            nsum = (
                pad[pr, 1:-1] + pad[pr - 1, 1:-1] + pad[pr + 1, 1:-1]
                + pad[pr, 0:-2] + pad[pr, 2:]
            )
            cnt += int((nsum == 5.0).sum())
        out.append(float(cnt))
    out.append(0.0)
    return out


def combine_stats(all_core_outs, valid_mask, targets, n_pairs=N_PAIRS):
    vm = (np.asarray(valid_mask, np.float32).reshape(-1) >= 0.5).astype(np.float64)
    tg = np.asarray(targets).reshape(-1, H0, H0)
    n_total = vm.shape[0]
    per = np.zeros((N_SCALES, n_total), np.float64)
    for core, st in enumerate(all_core_outs):
        st = np.asarray(st, np.float64).sum(axis=0)  # reduce 128 partitions
        for j in range(n_pairs):
            g = core * n_pairs + j
            c = st[j * COLS_PER_PAIR : (j + 1) * COLS_PER_PAIR]
            seam = seam_interior_counts(tg[g])
            tgg = tg[g].astype(np.float64)
            host_S = [tgg.sum(), tgg[::2, ::2].sum(), tgg[::4, ::4].sum()]
            # s2 interior fully host-side (targets-only bookkeeping)
            t2 = tgg[::4, ::4]
            pad = np.pad(t2, 1)
            nsum2 = (pad[1:-1, 1:-1] + pad[:-2, 1:-1] + pad[2:, 1:-1]
                     + pad[1:-1, :-2] + pad[1:-1, 2:])
            int2_host = float(((nsum2 == 5.0) & (t2 == 1.0)).sum())
            interior = [c[C_CNT0A] + c[C_CNT0B], c[C_CNT1], int2_host]
            inter_v = [c[C_INT0A] + c[C_INT0B], c[C_INT1], c[C_INT2]]
            z_v = [c[C_Z0A] + c[C_Z0B], c[C_Z1], c[C_Z2]]
            for s in range(N_SCALES):
                S = host_S[s]
                C = S - (interior[s] + seam[s])
                alpha = min(2.0 * (1.0 - (C + SMOOTH) / (S + SMOOTH)) - 1.0, 0.8)
                dou = (z_v[s] + S - 2.0 * inter_v[s] + SMOOTH) / (
                    z_v[s] + S - (1.0 + alpha) * inter_v[s] + SMOOTH
                )
                per[s, g] = dou if S > 0 else 0.0
    cnt = vm.sum()
    ws = np.array([1.0, 0.5, 0.25])
    ws = ws / ws.sum()
    loss = 0.0
    for s in range(N_SCALES):
        ls = (per[s] * vm).sum() / cnt if cnt > 0 else 0.0
        loss += ws[s] * ls
    return np.float32(loss)


def make_in_maps(inputs):
    l0 = np.ascontiguousarray(np.asarray(inputs["logits0"], np.float32).reshape(-1, H0, H0))
    l1 = np.ascontiguousarray(np.asarray(inputs["logits1"], np.float32).reshape(-1, H1, H1))
    l2 = np.ascontiguousarray(np.asarray(inputs["logits2"], np.float32).reshape(-1, H2, H2))
    tg = np.ascontiguousarray(np.asarray(inputs["targets"], np.int32).reshape(-1, H0, H0))
    consts = np.asarray(make_consts())
    in_maps = []
    for core in range(N_CORES):
        lo, hi = core * N_PAIRS, (core + 1) * N_PAIRS
        in_maps.append({
            "logits0": np.ascontiguousarray(l0[lo:hi]),
            "logits1": np.ascontiguousarray(l1[lo:hi]),
            "logits2": np.ascontiguousarray(l2[lo:hi]),
            "targets": np.ascontiguousarray(tg[lo:hi]),
            "consts_f8": consts,
        })
    return in_maps


def run_cores(inputs, **spmd_kwargs):
    from concourse.bass_utils import run_bass_kernel_spmd

    nc = get_kernel()
    in_maps = make_in_maps(inputs)
    return run_bass_kernel_spmd(nc, in_maps, core_ids=list(range(N_CORES)), **spmd_kwargs)


def kernel(**inputs) -> np.ndarray:
    res = run_cores(inputs)
    outs = [res.results[c]["out"] for c in range(N_CORES)]
    return combine_stats(outs, inputs["valid_mask"], inputs["targets"])


# revision 15
# speedup vs baseline: 1.1938x; 1.0045x over previous
"""Trainium2 Bass kernel for nn_DeepSupervisionBoundaryDoULoss.

kernel(**inputs) takes the FULL unsharded inputs (logits0/1/2, targets,
valid_mask) and returns the full scalar loss (float32).

Strategy: data-parallel over the 32 (b,n) pairs -> 4 pairs per core x 8 cores.
Each core streams its slice (~9.25 MiB) once; DMA is the roofline (~29us at
~340 GB/s/core). Pair-major pipeline; every queue carries the data in the
order compute consumes it (targets on the gpsimd SWDGE queue, logits0 on the
sync HWDGE queue, logits1+2 on the scalar HWDGE queue).

  - Pool(gpsimd): int32->fp8e4 casting DMAs for targets (deinterleaved rows
    A/B/C/D as [128, half, parity, 516] w/ 2-col zero pads), pad memsets.
  - ACT(scalar):  logits1/2 DMA issue (in the dead preamble window); sigmoid
    (f32->bf16, one op for merged s1+s2); z = sum(p^2) via Square w/ accum.
  - DVE(vector):  inter = sum(p*t) via scalar_tensor_tensor (bf16 x fp8);
    interior counts via stt-relu ((nsum-4) max 0) straight from PSUM with
    accum_out (one op over the 4-bank s0 PSUM + one merged s1+s2 op);
    t2 extraction copy.
  - PE(tensor):   3x3-cross conv as fp8 DoubleRow matmuls: per tile ONE DR
    fuses [I|band]@[center|other-parity] and ONE DR fuses [I|I]@[left|right]
    (overlapping strided ifmap APs), accumulated in PSUM f32.
  - Sync(SP):     consts + logits0 DMAs + per-pair stats out-DMAs.

All engine scratch is per-op-tagged so ACT and DVE never couple through
buffer reuse. Stats land in [128,10] f32 blocks per pair (partition-reduced
on host); the host finishes alpha/dou/weighted mean plus a seam correction
for the 4 rows/pair the on-chip conv cannot see (rows 255/256 at s0,
127/128 at s1), and splits the merged s1+s2 count using the (targets-only)
host s2 interior count.
"""

from contextlib import ExitStack

import numpy as np

N_PAIRS = 4
N_CORES = 8
H0, H1, H2 = 512, 256, 128
N_SCALES = 3
SMOOTH = 1e-5

# per-pair stats columns (s0 quantities split per half)
C_CNT0A, C_CNT0B, C_CNT1 = 0, 1, 2
C_INT0A, C_INT0B, C_INT1, C_INT2 = 3, 4, 5, 6
C_Z0A, C_Z0B, C_Z1, C_Z2 = 7, 8, 9, 10
COLS_PER_PAIR = 12

# consts layout (fp8), free-dim offsets
W_IB2M = 0      # [2,128]  DR weights [I | B2M]
W_B2PI = 256    # [2,128]  [B2P | I]
W_II = 512      # [2,128]  [I | I]
W_ITRI = 768    # [2,128]  [I | TRI]
W_I = 1024      # [128]    plain identity
W_EVEN = 1152   # [64]     even-partition selector
N_CONST = 1216

_NC_CACHE = {}


def make_consts():
    import ml_dtypes

    ident = np.eye(128, dtype=np.float32)
    b2m = np.zeros((128, 128), np.float32)  # q in {i-1, i}
    b2p = np.zeros((128, 128), np.float32)  # q in {i, i+1}
    tri = np.zeros((128, 128), np.float32)  # q in {i-1, i, i+1}
    for i in range(128):
        for dq in (-1, 0):
            if 0 <= i + dq < 128:
                b2m[i + dq, i] = 1.0
        for dq in (0, 1):
            if 0 <= i + dq < 128:
                b2p[i + dq, i] = 1.0
        for dq in (-1, 0, 1):
            if 0 <= i + dq < 128:
                tri[i + dq, i] = 1.0
    even = np.zeros((128, 64), np.float32)
    for i in range(64):
        even[2 * i, i] = 1.0
    c = np.zeros((128, N_CONST), np.float32)
    c[:, 0:128], c[:, 128:256] = ident, b2m
    c[:, 256:384], c[:, 384:512] = b2p, ident
    c[:, 512:640], c[:, 640:768] = ident, ident
    c[:, 768:896], c[:, 896:1024] = ident, tri
    c[:, 1024:1152] = ident
    c[:, 1152:1216] = even
    return c.astype(ml_dtypes.float8_e4m3fn)


def build_kernel(n_pairs=N_PAIRS):
    import concourse.tile as tile
    from concourse import bacc, mybir
    from bass_rust import AP

    F32 = mybir.dt.float32
    F8 = mybir.dt.float8e4
    BF16 = mybir.dt.bfloat16
    I32 = mybir.dt.int32
    ALU = mybir.AluOpType
    ACTF = mybir.ActivationFunctionType
    DRM = mybir.MatmulPerfMode.DoubleRow

    ncols = n_pairs * COLS_PER_PAIR
    nc = bacc.Bacc("TRN2", target_bir_lowering=False, debug=False)

    logits0 = nc.dram_tensor("logits0", [n_pairs, H0, H0], F32, kind="ExternalInput").ap()
    logits1 = nc.dram_tensor("logits1", [n_pairs, H1, H1], F32, kind="ExternalInput").ap()
    logits2 = nc.dram_tensor("logits2", [n_pairs, H2, H2], F32, kind="ExternalInput").ap()
    targets = nc.dram_tensor("targets", [n_pairs, H0, H0], I32, kind="ExternalInput").ap()
    consts8 = nc.dram_tensor("consts_f8", [128, N_CONST], F8, kind="ExternalInput").ap()
    out = nc.dram_tensor("out", [128, ncols], F32, kind="ExternalOutput").ap()

    with tile.TileContext(nc) as tc, ExitStack() as ctx:
        singles = ctx.enter_context(tc.tile_pool(name="singles", bufs=1))
        tpool = ctx.enter_context(tc.tile_pool(name="tpool", bufs=4))
        lpool = ctx.enter_context(tc.tile_pool(name="lpool", bufs=4))
        ppool = ctx.enter_context(tc.tile_pool(name="ppool", bufs=4))
        spool = ctx.enter_context(tc.tile_pool(name="spool", bufs=2))
        ps0p = ctx.enter_context(tc.tile_pool(name="ps0p", bufs=2, space="PSUM"))
        ps12p = ctx.enter_context(tc.tile_pool(name="ps12p", bufs=2, space="PSUM"))

        cb = singles.tile([128, N_CONST], F8)
        nc.sync.dma_start(out=cb, in_=consts8)
        stats = singles.tile([128, ncols], F32)
        nc.vector.memset(stats, 0.0)
        zeros1 = singles.tile([128, 1], BF16)
        nc.vector.memset(zeros1, 0.0)

        def wdr(off):
            return cb[:, off : off + 256].rearrange("p (two m) -> p two m", two=2)

        def windows(t, off, bstride, nb, istride, n):
            pstride = 1
            for s in t.tensor.shape[1:]:
                pstride *= s
            return AP(tensor=t.tensor, offset=off,
                      ap=[[pstride, 128], [bstride, nb], [istride, n]])

        # ---- all input DMAs queued up front, per-queue in consumption order
        l12s = []
        for p in range(n_pairs):
            l12 = lpool.tile([128, 640], F32, tag="l12", name=f"l12_{p}")
            nc.scalar.dma_start(
                out=l12[:, 0:512].rearrange("r (g c) -> r g c", g=2),
                in_=logits1[p].rearrange("(g r) c -> r g c", g=2),
            )
            nc.scalar.dma_start(out=l12[:, 512:640], in_=logits2[p])
            l12s.append(l12)
        t0s = []
        for p in range(n_pairs):
            t0 = tpool.tile([128, 2, 2, 516], F8, tag="t0", name=f"t0_{p}")
            for half in range(2):
                nc.gpsimd.dma_start(
                    out=t0[:, half, :, 2:514],
                    in_=targets[p, half * 256 : (half + 1) * 256].rearrange(
                        "(r parity) c -> r parity c", parity=2
                    ),
                )
            nc.gpsimd.memset(t0[:, :, :, 0:2], 0.0)
            nc.gpsimd.memset(t0[:, :, :, 514:516], 0.0)
            t0s.append(t0)
        l0s = []
        for p in range(n_pairs):
            l0 = lpool.tile([128, 2, 2, 512], F32, tag="l0", name=f"l0_{p}")
            for half in range(2):
                nc.sync.dma_start(
                    out=l0[:, half],
                    in_=logits0[p, half * 256 : (half + 1) * 256].rearrange(
                        "(r parity) c -> r parity c", parity=2
                    ),
                )
            l0s.append(l0)

        stcol = lambda p, c: stats[:, p * COLS_PER_PAIR + c : p * COLS_PER_PAIR + c + 1]

        p0s, p12s = [], []

        def emit_main(p):
            """Sigmoids + inter + conv + counts for pair p (no z-squares)."""
            t0, l0, l12 = t0s[p], l0s[p], l12s[p]

            # ---------------- scale 0 ----------------
            p0 = ppool.tile([128, 2, 2, 512], BF16, tag="p0", name=f"p0_{p}")
            p0s.append(p0)
            for half in range(2):
                nc.scalar.activation(out=p0[:, half], in_=l0[:, half],
                                     func=ACTF.Sigmoid)
            int0 = spool.tile([128, 2, 2, 512], BF16, tag="int0", name=f"int0_{p}")
            nc.vector.scalar_tensor_tensor(
                out=int0, in0=p0, scalar=1.0, in1=t0[:, :, :, 2:514],
                op0=ALU.mult, op1=ALU.mult, accum_out=stcol(p, C_INT0A),
            )
            for half in range(2):
                ps0 = ps0p.tile([128, 2, 512], F32, tag="ps0",
                                name=f"ps0_{p}_{half}")
                for parity in range(2):
                    dst = ps0[:, parity, :]
                    w1 = wdr(W_IB2M if parity == 0 else W_B2PI)
                    nc.tensor.matmul(dst, w1, t0[:, half, :, 2:514],
                                     start=True, stop=False, perf_mode=DRM)
                    off = (half * 2 + parity) * 516 + 1
                    nc.tensor.matmul(dst, wdr(W_II), windows(t0, off, 2, 2, 1, 512),
                                     start=False, stop=True, perf_mode=DRM)
                cnt0 = spool.tile([128, 2, 512], BF16, tag=f"cnt0{half}",
                                  name=f"cnt0_{p}_{half}")
                nc.vector.scalar_tensor_tensor(
                    out=cnt0, in0=ps0, scalar=-4.0,
                    in1=zeros1.broadcast_to([128, 2, 512]),
                    op0=ALU.add, op1=ALU.max, accum_out=stcol(p, C_CNT0A + half),
                )

            # ---------------- scales 1+2 ----------------
            p12 = ppool.tile([128, 640], BF16, tag="p12", name=f"p12_{p}")
            p12s.append(p12)
            nc.scalar.activation(out=p12, in_=l12, func=ACTF.Sigmoid)
            int1 = spool.tile([128, 2, 256], BF16, tag="int1", name=f"int1_{p}")
            nc.vector.scalar_tensor_tensor(
                out=int1, in0=p12[:, 0:512].rearrange("r (g c) -> r g c", g=2),
                scalar=1.0, in1=t0[:, :, 0, 2:514:2],
                op0=ALU.mult, op1=ALU.mult, accum_out=stcol(p, C_INT1),
            )
            # ps12: [0:512]=s1 conv, [512:640]=t2 extract. (The s2 interior
            # count is recovered on the host from targets, which it needs
            # anyway -- no on-device s2 conv.)
            ps12 = ps12p.tile([128, 640], F32, tag="ps12", name=f"ps12_{p}")
            for g in range(2):
                dst = ps12[:, g * 256 : (g + 1) * 256]
                off = (g * 2) * 516
                nc.tensor.matmul(dst, wdr(W_ITRI), windows(t0, off, 2, 2, 2, 256),
                                 start=True, stop=False, perf_mode=DRM)
                nc.tensor.matmul(dst, cb[:, W_I : W_I + 128],
                                 t0[:, g, 0, 4:516:2], start=False, stop=True)
            for g in range(2):
                nc.tensor.matmul(ps12[g * 64 : (g + 1) * 64, 512:640],
                                 cb[:, W_EVEN : W_EVEN + 64],
                                 t0[:, g, 0, 2:514:4], start=True, stop=True)
            t2 = tpool.tile([128, 128], F8, tag="t2", name=f"t2_{p}")
            nc.vector.tensor_copy(t2, ps12[:, 512:640])
            int2 = spool.tile([128, 128], BF16, tag="int2", name=f"int2_{p}")
            nc.vector.scalar_tensor_tensor(
                out=int2, in0=p12[:, 512:640], scalar=1.0, in1=t2,
                op0=ALU.mult, op1=ALU.mult, accum_out=stcol(p, C_INT2),
            )
            cnt1 = spool.tile([128, 512], BF16, tag="cnt1", name=f"cnt1_{p}")
            nc.vector.scalar_tensor_tensor(
                out=cnt1, in0=ps12[:, 0:512],
                scalar=-4.0, in1=zeros1.broadcast_to([128, 512]),
                op0=ALU.add, op1=ALU.max, accum_out=stcol(p, C_CNT1),
            )

        def emit_z(p):
            """z = sum(p^2) squares (deferred one pair) + stats out-DMA."""
            p0, p12 = p0s[p], p12s[p]
            sqr0 = spool.tile([128, 2, 2, 512], BF16, tag="sqr0", name=f"sqr0_{p}")
            nc.scalar.activation(out=sqr0, in_=p0, func=ACTF.Square,
                                 accum_out=stcol(p, C_Z0A))
            nc.scalar.activation(
                out=spool.tile([128, 512], BF16, tag="sqr1", name=f"sqr1_{p}"),
                in_=p12[:, 0:512], func=ACTF.Square, accum_out=stcol(p, C_Z1))
            nc.scalar.activation(
                out=spool.tile([128, 128], BF16, tag="sqr2", name=f"sqr2_{p}"),
                in_=p12[:, 512:640], func=ACTF.Square, accum_out=stcol(p, C_Z2))
            nc.sync.dma_start(
                out=out[:, p * COLS_PER_PAIR : p * COLS_PER_PAIR + 11],
                in_=stats[:, p * COLS_PER_PAIR : p * COLS_PER_PAIR + 11],
            )

        emit_main(0)
        emit_main(1)
        emit_z(0)
        emit_main(2)
        emit_z(1)
        emit_main(3)
        emit_z(2)
        emit_z(3)

    nc.compile()
    return nc


def get_kernel():
    if "nc" not in _NC_CACHE:
        _NC_CACHE["nc"] = build_kernel(N_PAIRS)
    return _NC_CACHE["nc"]


def seam_interior_counts(tg_pair):
    """Interior pixels in the seam rows the kernel cannot see (per scale)."""
    tg_pair = np.asarray(tg_pair)
    out = []
    for h in (H0, H1):
        step = H0 // h
        t = tg_pair[::step, ::step].astype(np.float64)
        pad = np.pad(t, 1)
        cnt = 0
        for r in (h // 2 - 1, h // 2):
            pr = r + 1
            nsum = (
                pad[pr, 1:-1] + pad[pr - 1, 1:-1] + pad[pr + 1, 1:-1]
                + pad[pr, 0:-2] + pad[pr, 2:]
            )
            cnt += int((nsum == 5.0).sum())
        out.append(float(cnt))
    out.append(0.0)
    return out


def combine_stats(all_core_outs, valid_mask, targets, n_pairs=N_PAIRS):
    vm = (np.asarray(valid_mask, np.float32).reshape(-1) >= 0.5).astype(np.float64)
    tg = np.asarray(targets).reshape(-1, H0, H0)
    n_total = vm.shape[0]
    per = np.zeros((N_SCALES, n_total), np.float64)
    for core, st in enumerate(all_core_outs):
        st = np.asarray(st, np.float64).sum(axis=0)  # reduce 128 partitions
        for j in range(n_pairs):
            g = core * n_pairs + j
            c = st[j * COLS_PER_PAIR : (j + 1) * COLS_PER_PAIR]
            seam = seam_interior_counts(tg[g])
            tgg = tg[g].astype(np.float64)
            host_S = [tgg.sum(), tgg[::2, ::2].sum(), tgg[::4, ::4].sum()]
            # s2 interior fully host-side (targets-only bookkeeping)
            t2 = tgg[::4, ::4]
            pad = np.pad(t2, 1)
            nsum2 = (pad[1:-1, 1:-1] + pad[:-2, 1:-1] + pad[2:, 1:-1]
                     + pad[1:-1, :-2] + pad[1:-1, 2:])
            int2_host = float(((nsum2 == 5.0) & (t2 == 1.0)).sum())
            interior = [c[C_CNT0A] + c[C_CNT0B], c[C_CNT1], int2_host]
            inter_v = [c[C_INT0A] + c[C_INT0B], c[C_INT1], c[C_INT2]]
            z_v = [c[C_Z0A] + c[C_Z0B], c[C_Z1], c[C_Z2]]
            for s in range(N_SCALES):
                S = host_S[s]
                C = S - (interior[s] + seam[s])
                alpha = min(2.0 * (1.0 - (C + SMOOTH) / (S + SMOOTH)) - 1.0, 0.8)
                dou = (z_v[s] + S - 2.0 * inter_v[s] + SMOOTH) / (
                    z_v[s] + S - (1.0 + alpha) * inter_v[s] + SMOOTH
                )
                per[s, g] = dou if S > 0 else 0.0
    cnt = vm.sum()
    ws = np.array([1.0, 0.5, 0.25])
    ws = ws / ws.sum()
    loss = 0.0
    for s in range(N_SCALES):
        ls = (per[s] * vm).sum() / cnt if cnt > 0 else 0.0
        loss += ws[s] * ls
    return np.float32(loss)


def make_in_maps(inputs):
    l0 = np.ascontiguousarray(np.asarray(inputs["logits0"], np.float32).reshape(-1, H0, H0))
    l1 = np.ascontiguousarray(np.asarray(inputs["logits1"], np.float32).reshape(-1, H1, H1))
    l2 = np.ascontiguousarray(np.asarray(inputs["logits2"], np.float32).reshape(-1, H2, H2))
    tg = np.ascontiguousarray(np.asarray(inputs["targets"], np.int32).reshape(-1, H0, H0))
    consts = np.asarray(make_consts())
    in_maps = []
    for core in range(N_CORES):
        lo, hi = core * N_PAIRS, (core + 1) * N_PAIRS
        in_maps.append({
            "logits0": np.ascontiguousarray(l0[lo:hi]),
            "logits1": np.ascontiguousarray(l1[lo:hi]),
            "logits2": np.ascontiguousarray(l2[lo:hi]),
            "targets": np.ascontiguousarray(tg[lo:hi]),
            "consts_f8": consts,
        })
    return in_maps


def run_cores(inputs, **spmd_kwargs):
    from concourse.bass_utils import run_bass_kernel_spmd

    nc = get_kernel()
    in_maps = make_in_maps(inputs)
    return run_bass_kernel_spmd(nc, in_maps, core_ids=list(range(N_CORES)), **spmd_kwargs)


def kernel(**inputs) -> np.ndarray:
    res = run_cores(inputs)
    outs = [res.results[c]["out"] for c in range(N_CORES)]
    return combine_stats(outs, inputs["valid_mask"], inputs["targets"])
